# revision 18
# baseline (speedup 1.0000x reference)
"""GCNConv (rank-1 normalized aggregation) Trainium2 kernel, SPMD over 8 cores.

Math (faithful to the torch/jax reference):
    h    = x @ W
    adj  = symmetric 0/1 adjacency from edge_index (duplicates collapse)
    deg  = adj.sum(1);  dinv = 1/sqrt(deg)
    agg  = dinv @ h = (dinv @ x) @ W          # rank-1 identity
    out  = dinv[:, None] * agg[None, :] + bias

Collectives here have ~55us fixed latency, so every core reads the full x
(3.07MB as bf16) and computes v = dinv @ x locally; only the O(N*F_OUT)
output is sharded across cores (1500 rows each).

v = dinv @ x runs on the Tensor engine, which otherwise idles during the
DMA stream: x ships as 24 tiles of 512 rows, [128 part, 4, 128] where
partition p holds rows {t*512 + a*128 + p}. Each tile is one matmul with
the STATIONARY operand lhsT = dinv4[t] ([128, 4]) and the tile as the
512-column moving operand, accumulating P[a, (b j)] in PSUM: the a==b
blocks are exactly the dinv-weighted row sums (1 cycle/node); a!=b blocks
are discarded. A short epilogue (one PSUM copy, 4 one-hot matmuls, a
transpose) assembles v as a [128, 1] column for agg = v @ W. All weights
are exact (no sorted-block approximation); error is just bf16 rounding.

Aux data ships as ONE packed constant DMA (dinv4 | W | dinvS | one-hots)
so the stream stays within the 8 shared DMA-completion semaphore lanes.

The exact deduplicated degree (integer/sorting work, not flops) is
computed on host with np.unique; all O(N*F) float math runs on device.
"""

import numpy as np

N, F_IN, F_OUT = 12000, 128, 256
N_CORES = 8
ROWS = N // N_CORES            # 1500 output rows per core
NT_OUT = 12                    # 12 row tiles per core (padded)
ROWS_PAD = NT_OUT * 128        # 1536

R = 4                          # row-groups per v-matmul tile
TROWS = R * 128                # 512 nodes per tile
NT_V = 24                      # v tiles (x padded to 12288 rows)
N_PAD = NT_V * TROWS           # 12288
# tiles per stream DMA; small final groups so the last matmuls trail the
# last DMA bytes by well under a microsecond
GRPS = [3, 3, 3, 3, 3, 3, 2, 2, 1, 1]
N_GRP = len(GRPS)              # 10 stream DMAs

# packed const layout (bf16/u16 elements)
CW_D4 = R * NT_V               # 96   dinv4 lhsT columns
CW_W = F_OUT                   # 256  weight
CW_DS = 2 * NT_OUT             # 24   f32 dinvS bit-packed
CW_E = R                       # 4    one-hot extraction vectors
CW = CW_D4 + CW_W + CW_DS + CW_E   # 380

OG_SIZES = [1, 2, 3, 3, 3]

_cache = {}


def _build_nc(zero_bias: bool):
    import concourse.bacc as bacc
    import concourse.mybir as mybir
    import concourse.tile as tile

    f32 = mybir.dt.float32
    bf16 = mybir.dt.bfloat16

    nc = bacc.Bacc(
        "TRN2",
        target_bir_lowering=False,
        debug=False,
        num_devices=N_CORES,
    )

    xg_d = [
        nc.dram_tensor(f"xg{i}", [128, g, R, F_IN], bf16, kind="ExternalInput")
        for i, g in enumerate(GRPS)
    ]
    cst_d = nc.dram_tensor("cst", [128, CW], bf16, kind="ExternalInput")
    if not zero_bias:
        b_d = nc.dram_tensor("bias", [F_OUT], f32, kind="ExternalInput")
    out_d = nc.dram_tensor("out", [ROWS_PAD, F_OUT], f32, kind="ExternalOutput")

    out_pnm = out_d.ap().rearrange("(n p) m -> p n m", p=128)  # [128,12,256]

    with tile.TileContext(nc) as tc:
        with (
            tc.tile_pool(name="const", bufs=1) as cpool,
            tc.tile_pool(name="xbuf", bufs=1) as xpool,
            tc.tile_pool(name="obuf", bufs=1) as opool,
            tc.tile_pool(name="ps", bufs=1, space="PSUM") as psum,
        ):
            # ---------------- DMA issue (per-queue FIFO order) -------------
            tg = [
                xpool.tile([128, g, R, F_IN], bf16, tag=f"tg{i}", name=f"tg{i}")
                for i, g in enumerate(GRPS)
            ]
            cst = cpool.tile([128, CW], bf16)

            # queue B (scalar) leads with the const; stream groups alternate
            nc.scalar.dma_start(cst[:], cst_d.ap())
            if not zero_bias:
                bias_s = cpool.tile([1, F_OUT], f32)
                nc.scalar.dma_start(
                    bias_s[:], b_d.ap().rearrange("(a n) -> a n", a=1)
                )
            for i in range(N_GRP):
                eng = nc.sync if i % 2 == 0 else nc.scalar
                eng.dma_start(tg[i][:], xg_d[i].ap())

            d4 = cst[:, 0:CW_D4]
            w_s = cst[:, CW_D4 : CW_D4 + CW_W]
            ds0 = CW_D4 + CW_W
            dinvS = cst[:, ds0 : ds0 + CW_DS].bitcast(f32)   # [128, 12]
            evec = cst[:, ds0 + CW_DS : CW]                  # one-hots rows 0..3

            if not zero_bias:
                ones1 = cpool.tile([1, 128], f32)
                nc.vector.memset(ones1[:], 1.0)
                pB2 = psum.tile([128, F_OUT], f32)
                nc.tensor.matmul(
                    pB2[:], ones1[:], bias_s[:], start=True, stop=True
                )
                B2 = cpool.tile([128, F_OUT], f32)
                nc.vector.tensor_copy(B2[:], pB2[:])

            # dummy matmuls (no stream deps) keep the PE HAM activity window
            # busy from kernel start so every real matmul runs at 2.4 GHz
            d_l = cpool.tile([1, 1], bf16)
            nc.vector.memset(d_l[:], 1.0)
            d_r = cpool.tile([1, 128], bf16)
            nc.vector.memset(d_r[:], 0.0)
            pdum = psum.tile([1, 128], f32)

            def warm():
                nc.tensor.matmul(
                    pdum[:], d_l[:], d_r[:], start=True, stop=True
                )

            # ---------------- v-reduction on TensorE -----------------------
            # P[a, (b j)] += sum_p dinv[t*512+a*128+p] * x[t*512+b*128+p, j]
            for _ in range(4):
                warm()
            P = psum.tile([R, R * F_IN], f32)
            t = 0
            for i, gn in enumerate(GRPS):
                for g in range(gn):
                    nc.tensor.matmul(
                        P[:],
                        d4[:, R * t : R * (t + 1)],
                        tg[i][:, g, :, :].rearrange("p a j -> p (a j)"),
                        start=(t == 0),
                        stop=(t == NT_V - 1),
                        skip_group_check=True,
                    )
                    t += 1

            # extract diagonal blocks as a column: with the S-block as the
            # STATIONARY operand, out = S_blk.T @ e_a = [128, 1] directly
            # (PSUM->SBUF copy split across DVE and ScalarE, in parallel)
            S = cpool.tile([R, R * F_IN], bf16)
            nc.vector.tensor_copy(S[:, 0 : 2 * F_IN], P[:, 0 : 2 * F_IN])
            nc.scalar.activation(
                S[:, 2 * F_IN : 4 * F_IN],
                P[:, 2 * F_IN : 4 * F_IN],
                mybir.ActivationFunctionType.Copy,
            )
            pvcol = psum.tile([F_IN, 1], f32)
            for a in range(R):
                nc.tensor.matmul(
                    pvcol[:],
                    S[:, a * F_IN : (a + 1) * F_IN],
                    evec[0:R, a : a + 1],
                    start=(a == 0),
                    stop=(a == R - 1),
                    skip_group_check=True,
                )
            vcol16 = cpool.tile([F_IN, 1], bf16)
            nc.vector.tensor_copy(vcol16[:], pvcol[:])

            # ---------------- agg = v @ W  (A2[p,o] = agg[o]) --------------
            pA2 = psum.tile([128, F_OUT], f32)
            nc.tensor.matmul(
                pA2[:],
                vcol16[:].broadcast_to([F_IN, 128]),
                w_s,
                start=True,
                stop=True,
            )
            A2 = cpool.tile([128, F_OUT], f32)

            # ---------------- out tiles: dinv_i*agg (+ bias) ---------------
            # ScalarE handles a few tiles straight from PSUM (parallel with
            # the DVE chain) so the out-DMA groups unblock sooner
            out_engines = [nc.sync, nc.scalar]
            ACT_TILES = {0, 8, 11} if zero_bias else set()
            ogs = {}
            base = 0
            for g, gsz in enumerate(OG_SIZES):
                ogs[g] = opool.tile(
                    [128, gsz, F_OUT], f32, tag=f"og{g}", name=f"og{g}"
                )
                base += gsz

            if zero_bias:
                base = 0
                for g, gsz in enumerate(OG_SIZES):
                    for j in range(gsz):
                        i = base + j
                        if i in ACT_TILES:
                            nc.scalar.activation(
                                ogs[g][:, j, :],
                                pA2[:],
                                mybir.ActivationFunctionType.Copy,
                                scale=dinvS[:, i : i + 1],
                            )
                    base += gsz
            nc.vector.tensor_copy(A2[:], pA2[:])
            base = 0
            for g, gsz in enumerate(OG_SIZES):
                og = ogs[g]
                for j in range(gsz):
                    i = base + j
                    if zero_bias:
                        if i in ACT_TILES:
                            continue
                        nc.vector.tensor_scalar(
                            og[:, j, :],
                            A2[:],
                            dinvS[:, i : i + 1],
                            None,
                            op0=mybir.AluOpType.mult,
                        )
                    else:
                        nc.vector.scalar_tensor_tensor(
                            og[:, j, :],
                            A2[:],
                            dinvS[:, i : i + 1],
                            B2[:],
                            op0=mybir.AluOpType.mult,
                            op1=mybir.AluOpType.add,
                        )
                out_engines[g % 2].dma_start(
                    out_pnm[:, base : base + gsz, :], og[:]
                )
                base += gsz

    nc.compile()
    return nc


def _get_nc(zero_bias: bool):
    key = ("nc", zero_bias)
    if key not in _cache:
        _cache[key] = _build_nc(zero_bias)
    return _cache[key]


def _host_dinv(edge_index: np.ndarray) -> np.ndarray:
    """Exact deduplicated symmetric degree -> 1/sqrt(deg), matching
    adj[a,b]=1; adj[b,a]=1; deg=adj.sum(1)."""
    a = edge_index[0].astype(np.int64)
    b = edge_index[1].astype(np.int64)
    keys = np.unique(np.concatenate([a * N + b, b * N + a]))
    deg = np.bincount(keys // N, minlength=N).astype(np.float32)
    with np.errstate(divide="ignore"):
        dinv = (np.float32(1.0) / np.sqrt(deg)).astype(np.float32)
    return dinv


def kernel(x, edge_index, weight, bias, _trace=False):
    from concourse import bass_utils
    import ml_dtypes

    bf16 = ml_dtypes.bfloat16

    x = np.ascontiguousarray(x, dtype=np.float32)
    weight = np.ascontiguousarray(weight, dtype=np.float32)
    bias = np.ascontiguousarray(bias, dtype=np.float32)
    dinv = _host_dinv(np.asarray(edge_index))

    zero_bias = not np.any(bias)
    nc = _get_nc(zero_bias)

    xp = np.zeros((N_PAD, F_IN), bf16)
    xp[:N] = x.astype(bf16)
    dp = np.zeros((N_PAD,), np.float32)
    dp[:N] = dinv

    # stream groups: xg[i][p, g, a, j] = xp[(t0+g)*512 + a*128 + p, j]
    xg4 = xp.reshape(NT_V, R, 128, F_IN)             # [t, a, p, j]
    im_shared = {}
    t0 = 0
    for i, gn in enumerate(GRPS):
        blk = xg4[t0 : t0 + gn]                      # [g, a, p, j]
        im_shared[f"xg{i}"] = np.ascontiguousarray(blk.transpose(2, 0, 1, 3))
        t0 += gn

    d4 = np.ascontiguousarray(
        dp.reshape(NT_V * R, 128).T.astype(bf16)     # [128, 96]
    )
    w16 = weight.astype(bf16)
    ev = np.zeros((128, R), bf16)
    for a in range(R):
        ev[a, a] = 1.0

    in_maps = []
    for c in range(N_CORES):
        r0 = c * ROWS
        ds = np.zeros((ROWS_PAD,), np.float32)
        ds[:ROWS] = dinv[r0 : r0 + ROWS]
        dinvS = np.ascontiguousarray(ds.reshape(NT_OUT, 128).T)  # [128, 12]
        cst = np.zeros((128, CW), bf16)
        cst[:, 0:CW_D4] = d4
        cst[:, CW_D4 : CW_D4 + CW_W] = w16
        ds0 = CW_D4 + CW_W
        cst[:, ds0 : ds0 + CW_DS] = dinvS.view(np.uint16).view(bf16)
        cst[:, ds0 + CW_DS : CW] = ev
        im = dict(im_shared)
        im["cst"] = cst
        if not zero_bias:
            im["bias"] = bias
        in_maps.append(im)

    res = bass_utils.run_bass_kernel_spmd(
        nc, in_maps, core_ids=list(range(N_CORES)), trace=_trace
    )
    out = np.concatenate(
        [res.results[c]["out"][:ROWS] for c in range(N_CORES)], axis=0
    )
    if _trace:
        _cache["last_results"] = res
    return out


# revision 19
# speedup vs baseline: 1.0319x; 1.0319x over previous
"""GCNConv (rank-1 normalized aggregation) Trainium2 kernel, SPMD over 8 cores.

Math (faithful to the torch/jax reference):
    h    = x @ W
    adj  = symmetric 0/1 adjacency from edge_index (duplicates collapse)
    deg  = adj.sum(1);  dinv = 1/sqrt(deg)
    agg  = dinv @ h = (dinv @ x) @ W          # rank-1 identity
    out  = dinv[:, None] * agg[None, :] + bias

Collectives here have ~55us fixed latency, so every core reads the full x
(3.07MB as bf16) and computes v = dinv @ x locally; only the O(N*F_OUT)
output is sharded across cores (1500 rows each).

v = dinv @ x runs on the Tensor engine, which otherwise idles during the
DMA stream: x ships as 24 tiles of 512 rows, [128 part, 4, 128] where
partition p holds rows {t*512 + a*128 + p}. Each tile is one matmul with
the STATIONARY operand lhsT = dinv4[t] ([128, 4]) and the tile as the
512-column moving operand, accumulating P[a, (b j)] in PSUM: the a==b
blocks are exactly the dinv-weighted row sums (1 cycle/node); a!=b blocks
are discarded. A short epilogue (one PSUM copy, 4 one-hot matmuls, a
transpose) assembles v as a [128, 1] column for agg = v @ W. All weights
are exact (no sorted-block approximation); error is just bf16 rounding.

Aux data ships as ONE packed constant DMA (dinv4 | W | dinvS | one-hots)
so the stream stays within the 8 shared DMA-completion semaphore lanes.

The exact deduplicated degree (integer/sorting work, not flops) is
computed on host with np.unique; all O(N*F) float math runs on device.
"""

import numpy as np

N, F_IN, F_OUT = 12000, 128, 256
N_CORES = 8
ROWS = N // N_CORES            # 1500 output rows per core
NT_OUT = 12                    # 12 row tiles per core (padded)
ROWS_PAD = NT_OUT * 128        # 1536

R = 4                          # row-groups per v-matmul tile
TROWS = R * 128                # 512 nodes per tile
NT_V = 24                      # v tiles (x padded to 12288 rows)
N_PAD = NT_V * TROWS           # 12288
# tiles per stream DMA
GRPS = [3, 3, 3, 3, 3, 3, 3, 3]
N_GRP = len(GRPS)              # 8 stream DMAs

# packed const layout (bf16/u16 elements)
CW_D4 = R * NT_V               # 96   dinv4 lhsT columns
CW_W = F_OUT                   # 256  weight
CW_DS = 2 * NT_OUT             # 24   f32 dinvS bit-packed
CW_E = R                       # 4    one-hot extraction vectors
CW = CW_D4 + CW_W + CW_DS + CW_E   # 380

OG_SIZES = [1, 2, 3, 3, 3]

_cache = {}


def _build_nc(zero_bias: bool):
    import concourse.bacc as bacc
    import concourse.mybir as mybir
    import concourse.tile as tile

    f32 = mybir.dt.float32
    bf16 = mybir.dt.bfloat16

    nc = bacc.Bacc(
        "TRN2",
        target_bir_lowering=False,
        debug=False,
        num_devices=N_CORES,
    )

    xg_d = [
        nc.dram_tensor(f"xg{i}", [128, g, R, F_IN], bf16, kind="ExternalInput")
        for i, g in enumerate(GRPS)
    ]
    cst_d = nc.dram_tensor("cst", [128, CW], bf16, kind="ExternalInput")
    if not zero_bias:
        b_d = nc.dram_tensor("bias", [F_OUT], f32, kind="ExternalInput")
    out_d = nc.dram_tensor("out", [ROWS_PAD, F_OUT], f32, kind="ExternalOutput")

    out_pnm = out_d.ap().rearrange("(n p) m -> p n m", p=128)  # [128,12,256]

    with tile.TileContext(nc) as tc:
        with (
            tc.tile_pool(name="const", bufs=1) as cpool,
            tc.tile_pool(name="xbuf", bufs=1) as xpool,
            tc.tile_pool(name="obuf", bufs=1) as opool,
            tc.tile_pool(name="ps", bufs=1, space="PSUM") as psum,
        ):
            # ---------------- DMA issue (per-queue FIFO order) -------------
            tg = [
                xpool.tile([128, g, R, F_IN], bf16, tag=f"tg{i}", name=f"tg{i}")
                for i, g in enumerate(GRPS)
            ]
            cst = cpool.tile([128, CW], bf16)

            # queue B (scalar) leads with the const; stream groups alternate
            nc.scalar.dma_start(cst[:], cst_d.ap())
            if not zero_bias:
                bias_s = cpool.tile([1, F_OUT], f32)
                nc.scalar.dma_start(
                    bias_s[:], b_d.ap().rearrange("(a n) -> a n", a=1)
                )
            for i in range(N_GRP):
                eng = nc.sync if i % 2 == 0 else nc.scalar
                eng.dma_start(tg[i][:], xg_d[i].ap())

            d4 = cst[:, 0:CW_D4]
            w_s = cst[:, CW_D4 : CW_D4 + CW_W]
            ds0 = CW_D4 + CW_W
            dinvS = cst[:, ds0 : ds0 + CW_DS].bitcast(f32)   # [128, 12]
            evec = cst[:, ds0 + CW_DS : CW]                  # one-hots rows 0..3

            if not zero_bias:
                ones1 = cpool.tile([1, 128], f32)
                nc.vector.memset(ones1[:], 1.0)
                pB2 = psum.tile([128, F_OUT], f32)
                nc.tensor.matmul(
                    pB2[:], ones1[:], bias_s[:], start=True, stop=True
                )
                B2 = cpool.tile([128, F_OUT], f32)
                nc.vector.tensor_copy(B2[:], pB2[:])

            # dummy matmuls (no stream deps) keep the PE HAM activity window
            # busy from kernel start so every real matmul runs at 2.4 GHz
            d_l = cpool.tile([1, 1], bf16)
            nc.vector.memset(d_l[:], 1.0)
            d_r = cpool.tile([1, 128], bf16)
            nc.vector.memset(d_r[:], 0.0)
            pdum = psum.tile([1, 128], f32)

            def warm():
                nc.tensor.matmul(
                    pdum[:], d_l[:], d_r[:], start=True, stop=True
                )

            # ---------------- v-reduction on TensorE -----------------------
            # P[a, (b j)] += sum_p dinv[t*512+a*128+p] * x[t*512+b*128+p, j]
            for _ in range(4):
                warm()
            P = psum.tile([R, R * F_IN], f32)
            t = 0
            for i, gn in enumerate(GRPS):
                for g in range(gn):
                    nc.tensor.matmul(
                        P[:],
                        d4[:, R * t : R * (t + 1)],
                        tg[i][:, g, :, :].rearrange("p a j -> p (a j)"),
                        start=(t == 0),
                        stop=(t == NT_V - 1),
                        skip_group_check=True,
                    )
                    t += 1

            # extract diagonal blocks as a column: with the S-block as the
            # STATIONARY operand, out = S_blk.T @ e_a = [128, 1] directly
            # (PSUM->SBUF copy split across DVE and ScalarE, in parallel)
            S = cpool.tile([R, R * F_IN], bf16)
            nc.vector.tensor_copy(S[:, 0 : 2 * F_IN], P[:, 0 : 2 * F_IN])
            nc.scalar.activation(
                S[:, 2 * F_IN : 4 * F_IN],
                P[:, 2 * F_IN : 4 * F_IN],
                mybir.ActivationFunctionType.Copy,
            )
            pvcol = psum.tile([F_IN, 1], f32)
            for a in range(R):
                nc.tensor.matmul(
                    pvcol[:],
                    S[:, a * F_IN : (a + 1) * F_IN],
                    evec[0:R, a : a + 1],
                    start=(a == 0),
                    stop=(a == R - 1),
                    skip_group_check=True,
                )
            vcol16 = cpool.tile([F_IN, 1], bf16)
            nc.vector.tensor_copy(vcol16[:], pvcol[:])

            # ---------------- agg = v @ W  (A2[p,o] = agg[o]) --------------
            pA2 = psum.tile([128, F_OUT], f32)
            nc.tensor.matmul(
                pA2[:],
                vcol16[:].broadcast_to([F_IN, 128]),
                w_s,
                start=True,
                stop=True,
            )
            A2 = cpool.tile([128, F_OUT], f32)

            # ---------------- out tiles: dinv_i*agg (+ bias) ---------------
            # ScalarE handles a few tiles straight from PSUM (parallel with
            # the DVE chain) so the out-DMA groups unblock sooner
            out_engines = [nc.sync, nc.scalar]
            ACT_TILES = {0, 8, 11} if zero_bias else set()
            ogs = {}
            base = 0
            for g, gsz in enumerate(OG_SIZES):
                ogs[g] = opool.tile(
                    [128, gsz, F_OUT], f32, tag=f"og{g}", name=f"og{g}"
                )
                base += gsz

            if zero_bias:
                base = 0
                for g, gsz in enumerate(OG_SIZES):
                    for j in range(gsz):
                        i = base + j
                        if i in ACT_TILES:
                            nc.scalar.activation(
                                ogs[g][:, j, :],
                                pA2[:],
                                mybir.ActivationFunctionType.Copy,
                                scale=dinvS[:, i : i + 1],
                            )
                    base += gsz
            nc.vector.tensor_copy(A2[:], pA2[:])
            base = 0
            for g, gsz in enumerate(OG_SIZES):
                og = ogs[g]
                for j in range(gsz):
                    i = base + j
                    if zero_bias:
                        if i in ACT_TILES:
                            continue
                        nc.vector.tensor_scalar(
                            og[:, j, :],
                            A2[:],
                            dinvS[:, i : i + 1],
                            None,
                            op0=mybir.AluOpType.mult,
                        )
                    else:
                        nc.vector.scalar_tensor_tensor(
                            og[:, j, :],
                            A2[:],
                            dinvS[:, i : i + 1],
                            B2[:],
                            op0=mybir.AluOpType.mult,
                            op1=mybir.AluOpType.add,
                        )
                out_engines[g % 2].dma_start(
                    out_pnm[:, base : base + gsz, :], og[:]
                )
                base += gsz

    nc.compile()
    return nc


def _get_nc(zero_bias: bool):
    key = ("nc", zero_bias)
    if key not in _cache:
        _cache[key] = _build_nc(zero_bias)
    return _cache[key]


def _host_dinv(edge_index: np.ndarray) -> np.ndarray:
    """Exact deduplicated symmetric degree -> 1/sqrt(deg), matching
    adj[a,b]=1; adj[b,a]=1; deg=adj.sum(1)."""
    a = edge_index[0].astype(np.int64)
    b = edge_index[1].astype(np.int64)
    keys = np.unique(np.concatenate([a * N + b, b * N + a]))
    deg = np.bincount(keys // N, minlength=N).astype(np.float32)
    with np.errstate(divide="ignore"):
        dinv = (np.float32(1.0) / np.sqrt(deg)).astype(np.float32)
    return dinv


def kernel(x, edge_index, weight, bias, _trace=False):
    from concourse import bass_utils
    import ml_dtypes

    bf16 = ml_dtypes.bfloat16

    x = np.ascontiguousarray(x, dtype=np.float32)
    weight = np.ascontiguousarray(weight, dtype=np.float32)
    bias = np.ascontiguousarray(bias, dtype=np.float32)
    dinv = _host_dinv(np.asarray(edge_index))

    zero_bias = not np.any(bias)
    nc = _get_nc(zero_bias)

    xp = np.zeros((N_PAD, F_IN), bf16)
    xp[:N] = x.astype(bf16)
    dp = np.zeros((N_PAD,), np.float32)
    dp[:N] = dinv

    # stream groups: xg[i][p, g, a, j] = xp[(t0+g)*512 + a*128 + p, j]
    xg4 = xp.reshape(NT_V, R, 128, F_IN)             # [t, a, p, j]
    im_shared = {}
    t0 = 0
    for i, gn in enumerate(GRPS):
        blk = xg4[t0 : t0 + gn]                      # [g, a, p, j]
        im_shared[f"xg{i}"] = np.ascontiguousarray(blk.transpose(2, 0, 1, 3))
        t0 += gn

    d4 = np.ascontiguousarray(
        dp.reshape(NT_V * R, 128).T.astype(bf16)     # [128, 96]
    )
    w16 = weight.astype(bf16)
    ev = np.zeros((128, R), bf16)
    for a in range(R):
        ev[a, a] = 1.0

    in_maps = []
    for c in range(N_CORES):
        r0 = c * ROWS
        ds = np.zeros((ROWS_PAD,), np.float32)
        ds[:ROWS] = dinv[r0 : r0 + ROWS]
        dinvS = np.ascontiguousarray(ds.reshape(NT_OUT, 128).T)  # [128, 12]
        cst = np.zeros((128, CW), bf16)
        cst[:, 0:CW_D4] = d4
        cst[:, CW_D4 : CW_D4 + CW_W] = w16
        ds0 = CW_D4 + CW_W
        cst[:, ds0 : ds0 + CW_DS] = dinvS.view(np.uint16).view(bf16)
        cst[:, ds0 + CW_DS : CW] = ev
        im = dict(im_shared)
        im["cst"] = cst
        if not zero_bias:
            im["bias"] = bias
        in_maps.append(im)

    res = bass_utils.run_bass_kernel_spmd(
        nc, in_maps, core_ids=list(range(N_CORES)), trace=_trace
    )
    out = np.concatenate(
        [res.results[c]["out"][:ROWS] for c in range(N_CORES)], axis=0
    )
    if _trace:
        _cache["last_results"] = res
    return out


# revision 22
# speedup vs baseline: 1.0368x; 1.0047x over previous
"""GCNConv (rank-1 normalized aggregation) Trainium2 kernel, SPMD over 8 cores.

Math (faithful to the torch/jax reference):
    h    = x @ W
    adj  = symmetric 0/1 adjacency from edge_index (duplicates collapse)
    deg  = adj.sum(1);  dinv = 1/sqrt(deg)
    agg  = dinv @ h = (dinv @ x) @ W          # rank-1 identity
    out  = dinv[:, None] * agg[None, :] + bias

Collectives here have ~55us fixed latency, so every core reads the full x
(3.07MB as bf16) and computes v = dinv @ x locally; only the O(N*F_OUT)
output is sharded across cores (1500 rows each).

v = dinv @ x runs on the Tensor engine, which otherwise idles during the
DMA stream: x ships as 24 tiles of 512 rows, [128 part, 4, 128] where
partition p holds rows {t*512 + a*128 + p}. Each tile is one matmul with
the STATIONARY operand lhsT = dinv4[t] ([128, 4]) and the tile as the
512-column moving operand, accumulating P[a, (b j)] in PSUM: the a==b
blocks are exactly the dinv-weighted row sums (1 cycle/node); a!=b blocks
are discarded. A short epilogue (one PSUM copy, 4 one-hot matmuls, a
transpose) assembles v as a [128, 1] column for agg = v @ W. All weights
are exact (no sorted-block approximation); error is just bf16 rounding.

Aux data ships as ONE packed constant DMA (dinv4 | W | dinvS | one-hots)
so the stream stays within the 8 shared DMA-completion semaphore lanes.

The exact deduplicated degree (integer/sorting work, not flops) is
computed on host with np.unique; all O(N*F) float math runs on device.
"""

import numpy as np

N, F_IN, F_OUT = 12000, 128, 256
N_CORES = 8
ROWS = N // N_CORES            # 1500 output rows per core
NT_OUT = 12                    # 12 row tiles per core (padded)
ROWS_PAD = NT_OUT * 128        # 1536

R = 4                          # row-groups per v-matmul tile
TROWS = R * 128                # 512 nodes per tile
NT_V = 24                      # v tiles (x padded to 12288 rows)
N_PAD = NT_V * TROWS           # 12288
# tiles per stream DMA
GRPS = [3, 3, 3, 3, 3, 3, 3, 3]
N_GRP = len(GRPS)              # 8 stream DMAs

# packed const layout (bf16/u16 elements)
CW_D4 = R * NT_V               # 96   dinv4 lhsT columns
CW_W = F_OUT                   # 256  weight
CW_DS = 2 * NT_OUT             # 24   f32 dinvS bit-packed
CW_E = R                       # 4    one-hot extraction vectors
CW = CW_D4 + CW_W + CW_DS + CW_E   # 380

OG_SIZES = [1, 1, 2, 2, 3, 3]

_cache = {}


def _build_nc(zero_bias: bool):
    import concourse.bacc as bacc
    import concourse.mybir as mybir
    import concourse.tile as tile

    f32 = mybir.dt.float32
    bf16 = mybir.dt.bfloat16

    nc = bacc.Bacc(
        "TRN2",
        target_bir_lowering=False,
        debug=False,
        num_devices=N_CORES,
    )

    xg_d = [
        nc.dram_tensor(f"xg{i}", [128, g, R, F_IN], bf16, kind="ExternalInput")
        for i, g in enumerate(GRPS)
    ]
    cst_d = nc.dram_tensor("cst", [128, CW], bf16, kind="ExternalInput")
    if not zero_bias:
        b_d = nc.dram_tensor("bias", [F_OUT], f32, kind="ExternalInput")
    out_d = nc.dram_tensor("out", [ROWS_PAD, F_OUT], f32, kind="ExternalOutput")

    out_pnm = out_d.ap().rearrange("(n p) m -> p n m", p=128)  # [128,12,256]

    with tile.TileContext(nc) as tc:
        with (
            tc.tile_pool(name="const", bufs=1) as cpool,
            tc.tile_pool(name="xbuf", bufs=1) as xpool,
            tc.tile_pool(name="obuf", bufs=1) as opool,
            tc.tile_pool(name="ps", bufs=1, space="PSUM") as psum,
        ):
            # ---------------- DMA issue (per-queue FIFO order) -------------
            tg = [
                xpool.tile([128, g, R, F_IN], bf16, tag=f"tg{i}", name=f"tg{i}")
                for i, g in enumerate(GRPS)
            ]
            cst = cpool.tile([128, CW], bf16)

            # queue B (scalar) leads with the const; stream groups alternate
            nc.scalar.dma_start(cst[:], cst_d.ap())
            if not zero_bias:
                bias_s = cpool.tile([1, F_OUT], f32)
                nc.scalar.dma_start(
                    bias_s[:], b_d.ap().rearrange("(a n) -> a n", a=1)
                )
            for i in range(N_GRP):
                eng = nc.sync if i % 2 == 0 else nc.scalar
                eng.dma_start(tg[i][:], xg_d[i].ap())

            d4 = cst[:, 0:CW_D4]
            w_s = cst[:, CW_D4 : CW_D4 + CW_W]
            ds0 = CW_D4 + CW_W
            dinvS = cst[:, ds0 : ds0 + CW_DS].bitcast(f32)   # [128, 12]
            evec = cst[:, ds0 + CW_DS : CW]                  # one-hots rows 0..3

            if not zero_bias:
                ones1 = cpool.tile([1, 128], f32)
                nc.vector.memset(ones1[:], 1.0)
                pB2 = psum.tile([128, F_OUT], f32)
                nc.tensor.matmul(
                    pB2[:], ones1[:], bias_s[:], start=True, stop=True
                )
                B2 = cpool.tile([128, F_OUT], f32)
                nc.vector.tensor_copy(B2[:], pB2[:])

            # dummy matmuls (no stream deps) keep the PE HAM activity window
            # busy from kernel start so every real matmul runs at 2.4 GHz
            d_l = cpool.tile([1, 1], bf16)
            nc.vector.memset(d_l[:], 1.0)
            d_r = cpool.tile([1, 128], bf16)
            nc.vector.memset(d_r[:], 0.0)
            pdum = psum.tile([1, 128], f32)

            def warm():
                nc.tensor.matmul(
                    pdum[:], d_l[:], d_r[:], start=True, stop=True
                )

            # ---------------- v-reduction on TensorE -----------------------
            # P[a, (b j)] += sum_p dinv[t*512+a*128+p] * x[t*512+b*128+p, j]
            for _ in range(4):
                warm()
            P = psum.tile([R, R * F_IN], f32)
            t = 0
            for i, gn in enumerate(GRPS):
                for g in range(gn):
                    nc.tensor.matmul(
                        P[:],
                        d4[:, R * t : R * (t + 1)],
                        tg[i][:, g, :, :].rearrange("p a j -> p (a j)"),
                        start=(t == 0),
                        stop=(t == NT_V - 1),
                        skip_group_check=True,
                    )
                    t += 1

            # extract diagonal blocks as a column: with the S-block as the
            # STATIONARY operand, out = S_blk.T @ e_a = [128, 1] directly
            S = cpool.tile([R, R * F_IN], bf16)
            nc.vector.tensor_copy(S[:], P[:])
            pvcol = psum.tile([F_IN, 1], f32)
            for a in range(R):
                nc.tensor.matmul(
                    pvcol[:],
                    S[:, a * F_IN : (a + 1) * F_IN],
                    evec[0:R, a : a + 1],
                    start=(a == 0),
                    stop=(a == R - 1),
                    skip_group_check=True,
                )
            vcol16 = cpool.tile([F_IN, 1], bf16)
            nc.vector.tensor_copy(vcol16[:], pvcol[:])

            # ---------------- agg = v @ W  (A2[p,o] = agg[o]) --------------
            pA2 = psum.tile([128, F_OUT], f32)
            nc.tensor.matmul(
                pA2[:],
                vcol16[:].broadcast_to([F_IN, 128]),
                w_s,
                start=True,
                stop=True,
            )
            A2 = cpool.tile([128, F_OUT], f32)

            # ---------------- out tiles: dinv_i*agg (+ bias) ---------------
            out_engines = [nc.sync, nc.scalar]
            base = 0
            for g, gsz in enumerate(OG_SIZES):
                og = opool.tile([128, gsz, F_OUT], f32, tag=f"og{g}", name=f"og{g}")
                for j in range(gsz):
                    i = base + j
                    if zero_bias:
                        if i == 0:
                            # first tile straight from PSUM on the Scalar
                            # engine: skips the A2 copy on the latency path
                            nc.scalar.activation(
                                og[:, j, :],
                                pA2[:],
                                mybir.ActivationFunctionType.Copy,
                                scale=dinvS[:, 0:1],
                            )
                            nc.vector.tensor_copy(A2[:], pA2[:])
                        else:
                            nc.vector.tensor_scalar(
                                og[:, j, :],
                                A2[:],
                                dinvS[:, i : i + 1],
                                None,
                                op0=mybir.AluOpType.mult,
                            )
                    else:
                        if i == 0:
                            nc.vector.tensor_copy(A2[:], pA2[:])
                        nc.vector.scalar_tensor_tensor(
                            og[:, j, :],
                            A2[:],
                            dinvS[:, i : i + 1],
                            B2[:],
                            op0=mybir.AluOpType.mult,
                            op1=mybir.AluOpType.add,
                        )
                out_engines[g % 2].dma_start(
                    out_pnm[:, base : base + gsz, :], og[:]
                )
                base += gsz

    nc.compile()
    return nc


def _get_nc(zero_bias: bool):
    key = ("nc", zero_bias)
    if key not in _cache:
        _cache[key] = _build_nc(zero_bias)
    return _cache[key]


def _host_dinv(edge_index: np.ndarray) -> np.ndarray:
    """Exact deduplicated symmetric degree -> 1/sqrt(deg), matching
    adj[a,b]=1; adj[b,a]=1; deg=adj.sum(1)."""
    a = edge_index[0].astype(np.int64)
    b = edge_index[1].astype(np.int64)
    keys = np.unique(np.concatenate([a * N + b, b * N + a]))
    deg = np.bincount(keys // N, minlength=N).astype(np.float32)
    with np.errstate(divide="ignore"):
        dinv = (np.float32(1.0) / np.sqrt(deg)).astype(np.float32)
    return dinv


def kernel(x, edge_index, weight, bias, _trace=False):
    from concourse import bass_utils
    import ml_dtypes

    bf16 = ml_dtypes.bfloat16

    x = np.ascontiguousarray(x, dtype=np.float32)
    weight = np.ascontiguousarray(weight, dtype=np.float32)
    bias = np.ascontiguousarray(bias, dtype=np.float32)
    dinv = _host_dinv(np.asarray(edge_index))

    zero_bias = not np.any(bias)
    nc = _get_nc(zero_bias)

    xp = np.zeros((N_PAD, F_IN), bf16)
    xp[:N] = x.astype(bf16)
    dp = np.zeros((N_PAD,), np.float32)
    dp[:N] = dinv

    # stream groups: xg[i][p, g, a, j] = xp[(t0+g)*512 + a*128 + p, j]
    xg4 = xp.reshape(NT_V, R, 128, F_IN)             # [t, a, p, j]
    im_shared = {}
    t0 = 0
    for i, gn in enumerate(GRPS):
        blk = xg4[t0 : t0 + gn]                      # [g, a, p, j]
        im_shared[f"xg{i}"] = np.ascontiguousarray(blk.transpose(2, 0, 1, 3))
        t0 += gn

    d4 = np.ascontiguousarray(
        dp.reshape(NT_V * R, 128).T.astype(bf16)     # [128, 96]
    )
    w16 = weight.astype(bf16)
    ev = np.zeros((128, R), bf16)
    for a in range(R):
        ev[a, a] = 1.0

    in_maps = []
    for c in range(N_CORES):
        r0 = c * ROWS
        ds = np.zeros((ROWS_PAD,), np.float32)
        ds[:ROWS] = dinv[r0 : r0 + ROWS]
        dinvS = np.ascontiguousarray(ds.reshape(NT_OUT, 128).T)  # [128, 12]
        cst = np.zeros((128, CW), bf16)
        cst[:, 0:CW_D4] = d4
        cst[:, CW_D4 : CW_D4 + CW_W] = w16
        ds0 = CW_D4 + CW_W
        cst[:, ds0 : ds0 + CW_DS] = dinvS.view(np.uint16).view(bf16)
        cst[:, ds0 + CW_DS : CW] = ev
        im = dict(im_shared)
        im["cst"] = cst
        if not zero_bias:
            im["bias"] = bias
        in_maps.append(im)

    res = bass_utils.run_bass_kernel_spmd(
        nc, in_maps, core_ids=list(range(N_CORES)), trace=_trace
    )
    out = np.concatenate(
        [res.results[c]["out"][:ROWS] for c in range(N_CORES)], axis=0
    )
    if _trace:
        _cache["last_results"] = res
    return out
